# revision 14
# baseline (speedup 1.0000x reference)
"""Depth rasterization (MANO hand z-buffer @ 640x640 -> bilinear 128x128).

Key identities exploited:
  * jax.image.resize(640->128, linear, antialias=False) samples input coords
    5*j + 2.0 exactly -> output[i, j] == raster[5i+2, 5j+2]. Only the 128x128
    decimated pixel grid (centers x = 5j+2.5, y = 5i+2.5) is rasterized: a
    25x reduction vs the reference's 640x640 raster.
  * Edge functions and barycentric depth are affine in pixel coords, so each
    triangle yields four planes over the basis (j, i, 1):
      P_k = OFF - S * sign(area) * e_k     (k = 0,1,2 penalty planes)
      W   = (e0*z0 + e1*z1 + e2*z2) / area (depth plane)
    key(p, f) = max(P0, P1, P2, W) equals the interpolated depth when p is
    inside triangle f and is >= OFF (>> the 100 clamp) outside; the z-buffer
    is zbuf(p) = min(100, min_f key(p, f)).
  * Plane evaluation is a K=9 bf16 matmul (coefficients split into 3 bf16
    limbs; the (j, i, 1) basis is exact in bf16, giving fp32-grade accuracy
    at full bf16 PE speed); the 4 planes run concurrently in the PE via
    32-row tile_position groups.
  * Per 16x8-pixel tile, candidates are bbox-filtered and hierarchical-z
    pruned on the host (exact: a candidate whose minimum possible depth over
    the tile exceeds the best fully-covering candidate's maximum depth can
    never win). Tiles are assigned to kernel "slots" sorted by candidate
    count; slot capacities (compile-time) are the per-rank maxima across all
    8 cores, so every tile fits exactly - no truncation for any input.

Sharding: 8 cores = 4 batch elements x 2 half-images (64 tiles of 16x8 px).
"""

import numpy as np
import ml_dtypes

import concourse.bacc as bacc
import concourse.mybir as mybir
import concourse.tile as tile
from concourse.bass_utils import run_bass_kernel_spmd

_B, _V, _F = 4, 778, 1538
_H = _W = 128
_NT = 64           # tiles (slots) per core
_TJ, _TI = 16, 8   # tile size in output pixels (x, y)
_OFF = 1000.0      # penalty-plane offset (>> 100 clamp)
_S = 1.0e9         # penalty scale
_BIGC = 1.0e7      # plane constant for padding/invalid
_CLAMP = 100.0
_COVER_MARGIN = 1.0    # e*s margin (e-units) for the full-cover test
_BOUND_MARGIN = 1e-3   # depth margin for the prune bound

_F32 = mybir.dt.float32
_BF16 = mybir.dt.bfloat16
_BF16_NP = ml_dtypes.bfloat16

_NC_CACHE = {}
PROFILE = {}


def _build_nc(caps, groups):
    """caps: 64 slot widths (32-granular, <= 512); groups: ((w, k), ...) with
    k consecutive slots of width w per group, k*w <= 512."""
    total2 = 2 * int(sum(caps))
    nc = bacc.Bacc("TRN2", target_bir_lowering=False, debug=False, num_devices=8)
    # dense [128, ...] input: cols [0:total2] pair-merged coef streams
    # (comp-A = [P0|W] limbs at partitions 0-8 & 64-72, comp-B = [P1|P2] at
    # 32-40 & 96-104), then NT*128 pixel-basis cols at all four row-groups.
    data_d = nc.dram_tensor("data", [128, total2 + _NT * 128], _BF16, kind="ExternalInput")
    out_d = nc.dram_tensor("out", [128, _NT], _F32, kind="ExternalOutput")

    with tile.TileContext(nc) as tc:
        with (
            tc.tile_pool(name="const", bufs=1) as cpool,
            tc.tile_pool(name="scr", bufs=3) as spool,
            tc.tile_pool(name="ps", bufs=4, space="PSUM") as ppool,
        ):
            zmin = cpool.tile([128, _NT], _F32)
            # DMA in chunks: one coef tile per compute group (overlaps with
            # compute on earlier groups) + 8 pixel-basis chunk tiles
            ctiles = []
            off = 0
            for gi, (w, k) in enumerate(groups):
                kw2 = 2 * w * k
                ct = cpool.tile([128, kw2], _BF16, name=f"coef{gi}")
                nc.sync.dma_start(ct[:], data_d.ap()[:, off : off + kw2])
                ctiles.append(ct)
                off += kw2
            ptiles = []
            for g in range(8):
                pt = cpool.tile([128, 8 * 128], _BF16, name=f"pix{g}")
                nc.sync.dma_start(pt[:], data_d.ap()[:, total2 + g * 1024 : total2 + (g + 1) * 1024])
                ptiles.append(pt)

            gbase = 0
            for gi, (w, k) in enumerate(groups):
                kw = k * w
                kw2 = 2 * kw
                coeft = ctiles[gi]
                pa = ppool.tile([128, 1024], _F32, tag="ps", name="pa")
                pb = ppool.tile([128, 1024], _F32, tag="ps", name="pb")
                # matmuls: comp-A chunks alternate row-groups 0/64, comp-B 32/96
                for q in range(k):
                    s = gbase + q
                    pt = ptiles[s // 8]
                    cols = slice((s % 8) * 128, (s % 8) * 128 + 128)
                    o = 2 * w * q
                    # a matmul write may not cross a PSUM bank (512-col)
                    # boundary: split the slot's range on the absolute grid
                    edges = [o, o + 2 * w]
                    for bnd in range(512, kw2, 512):
                        if o < bnd < o + 2 * w:
                            edges.insert(-1, bnd)
                    for ci in range(len(edges) - 1):
                        c0, c1 = edges[ci], edges[ci + 1]
                        ra = 0 if ci % 2 == 0 else 64
                        rb = 32 if ci % 2 == 0 else 96
                        nc.tensor.matmul(pa[:, c0:c1], pt[ra : ra + 9, cols],
                                         coeft[ra : ra + 9, c0:c1],
                                         start=True, stop=True, tile_position=(ra, 0))
                        nc.tensor.matmul(pb[:, c0:c1], pt[rb : rb + 9, cols],
                                         coeft[rb : rb + 9, c0:c1],
                                         start=True, stop=True, tile_position=(rb, 0))
                # ScalarE pulls comp-A to SBUF (DVE reads max one PSUM operand)
                ta = spool.tile([128, 1024], _F32, tag="ta", name="ta")
                nc.scalar.copy(ta[:, :kw2], pa[:, :kw2])
                # u = [max(P0,P1) | max(W,P2)] in one wide op
                u = spool.tile([128, 1024], _F32, tag="u", name="u")
                nc.vector.tensor_tensor(u[:, :kw2], ta[:, :kw2], pb[:, :kw2],
                                        op=mybir.AluOpType.max)
                # key = max of the two halves, per slot
                useg = u[:, :kw2].rearrange("p (k t w) -> p k t w", t=2, w=w)
                keyt = spool.tile([128, 512], _F32, tag="key", name="keyt")
                kseg = keyt[:, :kw].rearrange("p (k w) -> p k w", w=w)
                nc.vector.tensor_tensor(kseg, useg[:, :, 0, :], useg[:, :, 1, :],
                                        op=mybir.AluOpType.max)
                nc.vector.tensor_reduce(zmin[:, gbase : gbase + k], kseg,
                                        axis=mybir.AxisListType.X, op=mybir.AluOpType.min)
                gbase += k

            zclamp = cpool.tile([128, _NT], _F32)
            nc.vector.tensor_scalar_min(zclamp[:], zmin[:], _CLAMP)
            nc.sync.dma_start(out_d.ap(), zclamp[:])

    nc.compile()
    return nc


def _get_nc(caps, groups):
    key = (caps, groups)
    if key not in _NC_CACHE:
        _NC_CACHE[key] = _build_nc(caps, groups)
    return _NC_CACHE[key]


def _planes64(vertices, faces):
    """Full-precision planes on basis (j, i, 1): [B, 4, 3, F] f64 + aux."""
    v64 = vertices.astype(np.float64)
    fidx = np.asarray(faces).astype(np.int64).reshape(-1)
    fv = v64[:, fidx, :].reshape(_B, _F, 3, 3)
    x0, y0, z0 = fv[:, :, 0, 0], fv[:, :, 0, 1], fv[:, :, 0, 2]
    x1, y1, z1 = fv[:, :, 1, 0], fv[:, :, 1, 1], fv[:, :, 1, 2]
    x2, y2, z2 = fv[:, :, 2, 0], fv[:, :, 2, 1], fv[:, :, 2, 2]

    # area exactly as the reference computes it (float32 ops)
    v32 = vertices.astype(np.float32)
    fv32 = v32[:, fidx, :].reshape(_B, _F, 3, 3)
    xa, ya = fv32[:, :, 0, 0], fv32[:, :, 0, 1]
    xb, yb = fv32[:, :, 1, 0], fv32[:, :, 1, 1]
    xc, yc = fv32[:, :, 2, 0], fv32[:, :, 2, 1]
    area32 = (xb - xa) * (yc - ya) - (yb - ya) * (xc - xa)
    s = np.sign(area32).astype(np.float64)
    valid = np.abs(area32) > 1e-12

    A0 = -(y2 - y1); B0 = x2 - x1; C0 = (y2 - y1) * x1 - (x2 - x1) * y1
    A1 = -(y0 - y2); B1 = x0 - x2; C1 = (y0 - y2) * x2 - (x0 - x2) * y2
    A2 = -(y1 - y0); B2 = x1 - x0; C2 = (y1 - y0) * x0 - (x1 - x0) * y0

    area64 = np.where(valid, area32.astype(np.float64), 1.0)
    Aw = (z0 * A0 + z1 * A1 + z2 * A2) / area64
    Bw = (z0 * B0 + z1 * B1 + z2 * B2) / area64
    Cw = (z0 * C0 + z1 * C1 + z2 * C2) / area64

    planes = np.zeros((_B, 4, 3, _F), np.float64)
    raw = [
        (-_S * s * A0, -_S * s * B0, _OFF - _S * s * C0),
        (-_S * s * A1, -_S * s * B1, _OFF - _S * s * C1),
        (-_S * s * A2, -_S * s * B2, _OFF - _S * s * C2),
        (Aw, Bw, Cw),
    ]
    for k, (a, b, c) in enumerate(raw):
        a = np.where(valid, a, 0.0)
        b = np.where(valid, b, 0.0)
        c = np.where(valid, c, _BIGC)
        # basis change px = 5j + 2.5, py = 5i + 2.5 -> (j, i, 1)
        planes[:, k, 0] = 5.0 * a
        planes[:, k, 1] = 5.0 * b
        planes[:, k, 2] = 2.5 * a + 2.5 * b + c

    xsmin = fv[..., 0].min(2); xsmax = fv[..., 0].max(2)
    ysmin = fv[..., 1].min(2); ysmax = fv[..., 1].max(2)
    zmin_tri = fv[..., 2].min(2)
    return planes, valid, xsmin, xsmax, ysmin, ysmax, zmin_tri


def _split3(c64):
    """[rows, n] f64 -> [3*rows, n] bf16 (hi/mid/lo limbs)."""
    hi = c64.astype(_BF16_NP).astype(np.float64)
    mid = (c64 - hi).astype(_BF16_NP).astype(np.float64)
    lo = (c64 - hi - mid).astype(_BF16_NP)
    return hi.astype(_BF16_NP), mid.astype(_BF16_NP), lo


def _prepare(vertices, faces):
    """Host binning/pruning/packing. Returns (caps, in_maps data, slot maps)."""
    planes, valid, xsmin, xsmax, ysmin, ysmax, zmin_tri = _planes64(vertices, faces)
    ntj = _W // _TJ

    kept_lists = []  # [core][slot_ordering later] per-tile candidate arrays
    for c in range(8):
        b, h = c // 2, c % 2
        P = planes[b]  # [4, 3, F]
        tiles = []
        for t in range(_NT):
            tj, ti = t % ntj, t // ntj
            j0, i0 = tj * _TJ, ti * _TI + 64 * h
            xlo, xhi = 5 * j0 + 2.5, 5 * (j0 + _TJ - 1) + 2.5
            ylo, yhi = 5 * i0 + 2.5, 5 * (i0 + _TI - 1) + 2.5
            cand = np.where(valid[b] & (xsmax[b] >= xlo) & (xsmin[b] <= xhi)
                            & (ysmax[b] >= ylo) & (ysmin[b] <= yhi))[0]
            if len(cand) == 0:
                tiles.append((t, np.empty(0, np.int64)))
                continue
            corners = np.array(
                [[j0, i0, 1], [j0 + _TJ - 1, i0, 1],
                 [j0, i0 + _TI - 1, 1], [j0 + _TJ - 1, i0 + _TI - 1, 1]],
                np.float64)
            Wc = corners @ P[3][:, cand]           # [4, nc]
            zlo = np.maximum(Wc.min(0), zmin_tri[b][cand])
            covers = np.ones(len(cand), bool)
            for k in range(3):
                Pc = corners @ P[k][:, cand]
                covers &= (Pc <= _OFF - _S * _COVER_MARGIN).all(axis=0)
            bound = (Wc.max(0)[covers].min() + _BOUND_MARGIN) if covers.any() else np.inf
            keep = zlo <= bound
            order = cand[keep][np.argsort(zlo[keep])]
            tiles.append((t, order))
        kept_lists.append(tiles)

    # sort each core's tiles by kept desc -> slots; per-rank max across cores
    slot_orders = []
    for c in range(8):
        order = sorted(range(_NT), key=lambda t: -len(kept_lists[c][t][1]))
        slot_orders.append(order)
    raw = []
    for s in range(_NT):
        m = max(len(kept_lists[c][slot_orders[c][s]][1]) for c in range(8))
        raw.append(max(32, ((m + 31) // 32) * 32))
    assert all(raw[i] >= raw[i + 1] for i in range(_NT - 1))
    # group consecutive slots (padded to the group's max width w) while
    # k*w <= 512, so each group's pair-merged PSUM tile is <= 1024 cols
    groups = []
    s = 0
    while s < _NT:
        w = min(512, raw[s])
        k = 1
        while s + k < _NT and (k + 1) * w <= 512:
            k += 1
        groups.append((w, k))
        s += k
    groups = tuple(groups)
    caps = []
    for w, k in groups:
        caps.extend([w] * k)
    caps = tuple(caps)
    total = sum(caps)

    in_maps = []
    for c in range(8):
        b, h = c // 2, c % 2
        # pair-merged layout: comp-A = [P0 | W], comp-B = [P1 | P2], each
        # slot occupying 2*w columns in its component stream
        total2 = 2 * total
        compA = np.zeros((3, total2), np.float64)
        compB = np.zeros((3, total2), np.float64)
        compA[2, :] = _BIGC
        compB[2, :] = _BIGC
        pix_g = np.zeros((3, _NT * 128), np.float32)
        off = 0
        for s in range(_NT):
            t = slot_orders[c][s]
            idx = kept_lists[c][t][1]
            n = len(idx)
            w = caps[s]
            compA[:, off : off + n] = planes[b, 0][:, idx]          # P0
            compA[:, off + w : off + w + n] = planes[b, 3][:, idx]  # W
            compA[2, off + w + n : off + 2 * w] = _CLAMP + 1.0      # pad W half
            compB[:, off : off + n] = planes[b, 1][:, idx]          # P1
            compB[:, off + w : off + w + n] = planes[b, 2][:, idx]  # P2
            off += 2 * w
            tj, ti = t % (_W // _TJ), t // (_W // _TJ)
            j0, i0 = tj * _TJ, ti * _TI + 64 * h
            jj = j0 + np.tile(np.arange(_TJ, dtype=np.float32), _TI)
            ii = i0 + np.repeat(np.arange(_TI, dtype=np.float32), _TJ)
            pix_g[0, s * 128 : (s + 1) * 128] = jj
            pix_g[1, s * 128 : (s + 1) * 128] = ii
            pix_g[2, s * 128 : (s + 1) * 128] = 1.0
        # limb splits -> 9 rows per component
        data = np.zeros((128, total2 + _NT * 128), _BF16_NP)
        for comp, bases in ((compA, (0, 64)), (compB, (32, 96))):
            hi, mid, lo = _split3(comp)
            for base in bases:
                data[base + 0 : base + 3, :total2] = hi
                data[base + 3 : base + 6, :total2] = mid
                data[base + 6 : base + 9, :total2] = lo
        pix16 = np.vstack([pix_g, pix_g, pix_g]).astype(_BF16_NP)  # 9 rows
        for base in (0, 32, 64, 96):
            data[base : base + 9, total2:] = pix16
        in_maps.append({"data": data})
    return caps, groups, in_maps, slot_orders


def kernel(vertices, faces):
    vertices = np.asarray(vertices)
    faces = np.asarray(faces)
    caps, groups, in_maps, slot_orders = _prepare(vertices, faces)

    nc = _get_nc(caps, groups)
    kw = dict(PROFILE.get("run_kwargs", {}))
    res = run_bass_kernel_spmd(nc, in_maps, list(range(8)), **kw)
    PROFILE["last_result"] = res

    ntj = _W // _TJ
    out = np.empty((_B, _H, _W), np.float32)
    for c in range(8):
        b, h = c // 2, c % 2
        z = res.results[c]["out"]  # [128, NT]
        for s in range(_NT):
            t = slot_orders[c][s]
            tj, ti = t % ntj, t // ntj
            j0, i0 = tj * _TJ, ti * _TI + 64 * h
            out[b, i0 : i0 + _TI, j0 : j0 + _TJ] = z[:, s].reshape(_TI, _TJ)
    return out


# revision 23
# speedup vs baseline: 1.5554x; 1.5554x over previous
"""Depth rasterization (MANO hand z-buffer @ 640x640 -> bilinear 128x128).

Key identities exploited:
  * jax.image.resize(640->128, linear, antialias=False) samples input coords
    5*j + 2.0 exactly -> output[i, j] == raster[5i+2, 5j+2]. Only the 128x128
    decimated pixel grid (centers x = 5j+2.5, y = 5i+2.5) is rasterized: a
    25x reduction vs the reference's 640x640 raster.
  * Edge functions and barycentric depth are affine in pixel coords, so each
    triangle yields four planes over the basis (j, i, 1):
      P_k = OFF - S * sign(area) * e_k     (k = 0,1,2 penalty planes)
      W   = (e0*z0 + e1*z1 + e2*z2) / area (depth plane)
    key(p, f) = max(P0, P1, P2, W) equals the interpolated depth when p is
    inside triangle f and is >= OFF (>> the 100 clamp) outside; the z-buffer
    is zbuf(p) = min(100, min_f key(p, f)).
  * Plane evaluation is a K=9 bf16 matmul (coefficients split into 3 bf16
    limbs; the (j, i, 1) basis is exact in bf16, giving fp32-grade accuracy
    at bf16 PE speed); planes are pair-merged as comp-A = [P0|W] and
    comp-B = [P1|P2] streams evaluated on alternating PE row-groups.
  * Per 16x8-pixel tile, candidates are bbox-filtered and hierarchical-z
    pruned on the host (exact: a candidate whose minimum possible depth over
    the tile exceeds the best fully-covering candidate's maximum depth can
    never win). Tiles are chunked to <=256 candidates per work item (host
    min-merges chunks), items are rank-parity balanced across each batch's
    two cores, and slot capacities are per-rank maxima across all 8 cores -
    exact for any input, no truncation.
  * DVE work per slot is 3 element passes: one wide tensor_tensor max
    (u = max(compA, compB)) and a custom fused DVE op
    (out = max(u_lo, u_hi); accum = min-reduce seeded at 100).

Sharding: 8 cores; each batch element's 128 tiles split across 2 cores.
"""

import numpy as np
import ml_dtypes

import concourse.bacc as bacc
import concourse.mybir as mybir
import concourse.tile as tile
from concourse.bass_utils import run_bass_kernel_spmd

_B, _V, _F = 4, 778, 1538
_H = _W = 128
_TJ, _TI = 16, 8   # tile size in output pixels (x, y)
_NTILE = (_H // _TI) * (_W // _TJ)  # 128 tiles per batch image
_WMAX = 256        # max slot width (pair-merged 2w <= 512 = one PSUM bank)
_OFF = 1000.0      # penalty-plane offset (>> 100 clamp)
_S = 1.0e9         # penalty scale
_BIGC = 1.0e7      # plane constant for padding/invalid
_CLAMP = 100.0
_COVER_MARGIN = 1.0    # e*s margin (e-units) for the full-cover test
_BOUND_MARGIN = 1e-3   # depth margin for the prune bound

_F32 = mybir.dt.float32
_BF16 = mybir.dt.bfloat16
_BF16_NP = ml_dtypes.bfloat16

_NC_CACHE = {}
_OP_CACHE = {}
PROFILE = {}


def _maxpair_minred_op():
    """Custom DVE op: out = max(in0, in1); accum_out = min(out) seeded s0."""
    if "op" in _OP_CACHE:
        return _OP_CACHE["op"]
    import concourse.dve_ops as dve_ops
    from concourse.dve_spec import C0, Spec, Src0, Src1, lower, maxx, minn
    from concourse.dve_table_gen import dve_ver_for
    from concourse.dve_uop import DveOpSpec

    name = "MAXPAIR_MINRED_ANT"
    for op in dve_ops.OPS:
        if op.name == name:
            _OP_CACHE["op"] = op
            return op
    spec = Spec(body=maxx(Src0, Src1), accum=minn, accum_init=C0)
    opcode = dve_ops._CUSTOM_DVE_ROW_BASE + len(dve_ops.OPS)
    assert opcode < 0x20
    dve_ops._SUB_OPCODE_FOR_NAME[name] = opcode
    ver = dve_ver_for("TRN2")
    sha = DveOpSpec(name=name, opcode=opcode, uops=lower(spec, ver=ver),
                    rd1_en=True).sha(ver)
    op = dve_ops.DveOp(name, spec, subdim=False, uops_sha={ver: sha})
    dve_ops.OPS.append(op)
    dve_ops.CUSTOM_DVE_SPECS[name] = spec
    _OP_CACHE["op"] = op
    return op


def _build_nc(caps, groups):
    """caps: per-slot widths w (32-granular, <= _WMAX); groups: ((w, k), ...)
    of consecutive equal-width slots with 2*k*w <= 512 (one PSUM bank)."""
    nslot = len(caps)
    total2 = 2 * int(sum(caps))
    op = _maxpair_minred_op()
    nc = bacc.Bacc("TRN2", target_bir_lowering=False, debug=False, num_devices=8)
    # dense [128, ...] input: pair-merged coef streams (comp-A = [P0|W] limbs
    # at partitions 0-8 & 64-72, comp-B = [P1|P2] at 32-40 & 96-104), then
    # nslot*128 pixel-basis cols at all four row-groups.
    data_d = nc.dram_tensor("data", [128, total2 + nslot * 128], _BF16, kind="ExternalInput")
    out_d = nc.dram_tensor("out", [128, nslot], _F32, kind="ExternalOutput")

    with tile.TileContext(nc) as tc:
        with (
            tc.tile_pool(name="const", bufs=1) as cpool,
            tc.tile_pool(name="scr", bufs=6) as spool,
            tc.tile_pool(name="ps", bufs=8, space="PSUM") as ppool,
        ):
            zmin = cpool.tile([128, nslot], _F32)
            # coef DMA in ~6 chunks at group boundaries; pix in 4 chunks
            goff = [0]
            for w, k in groups:
                goff.append(goff[-1] + 2 * w * k)
            # chunk boundaries (in groups): fine-grained early so the first
            # compute groups start as soon as their data lands
            gb = [0, 1, 2, 4, 6, 9, 13, 18, 24]
            gb = sorted({min(g, len(groups)) for g in gb} | {len(groups)})
            slot_of_group = [0]
            for w, k in groups:
                slot_of_group.append(slot_of_group[-1] + k)
            ctiles = []  # (col range, tile)
            ptiles = []  # (slot range, tile)
            dmas = []
            for i in range(len(gb) - 1):
                c0, c1 = goff[gb[i]], goff[gb[i + 1]]
                s0, s1 = slot_of_group[gb[i]], slot_of_group[gb[i + 1]]
                if c1 > c0:
                    ct = cpool.tile([128, c1 - c0], _BF16, name=f"coef{i}")
                    ctiles.append((c0, c1, ct))
                    dmas.append((ct, data_d.ap()[:, c0:c1]))
                if s1 > s0:
                    pt = cpool.tile([128, (s1 - s0) * 128], _BF16, name=f"pix{i}")
                    ptiles.append((s0, s1, pt))
                    dmas.append((pt, data_d.ap()[:, total2 + s0 * 128 : total2 + s1 * 128]))
            for dst, srcap in dmas:
                nc.sync.dma_start(dst[:], srcap)

            def coef_view(c0, c1):
                for t0, t1, ct in ctiles:
                    if t0 <= c0 and c1 <= t1:
                        return ct[:, c0 - t0 : c1 - t0]
                raise AssertionError((c0, c1))

            def pix_view(s):
                for s0, s1, pt in ptiles:
                    if s0 <= s < s1:
                        return pt[:, (s - s0) * 128 : (s - s0 + 1) * 128]
                raise AssertionError(s)

            gbase = 0
            for gi, (w, k) in enumerate(groups):
                kw2 = 2 * w * k
                go = goff[gi]
                pa = ppool.tile([128, 512], _F32, tag="ps", name="pa")
                pb = ppool.tile([128, 512], _F32, tag="ps", name="pb")
                for q in range(k):
                    s = gbase + q
                    o = 2 * w * q
                    ra, rb = (0, 32) if gi % 2 == 0 else (64, 96)
                    pv = pix_view(s)
                    cv = coef_view(go + o, go + o + 2 * w)
                    nc.tensor.matmul(pa[:, o : o + 2 * w], pv[ra : ra + 9, :],
                                     cv[ra : ra + 9, :],
                                     start=True, stop=True, tile_position=(ra, 0))
                    nc.tensor.matmul(pb[:, o : o + 2 * w], pv[rb : rb + 9, :],
                                     cv[rb : rb + 9, :],
                                     start=True, stop=True, tile_position=(rb, 0))
                # ScalarE pulls comp-A to SBUF (DVE reads max one PSUM operand)
                ta = spool.tile([128, 512], _F32, tag="ta", name="ta")
                nc.scalar.copy(ta[:, :kw2], pa[:, :kw2])
                u = spool.tile([128, 512], _F32, tag="u", name="u")
                nc.vector.tensor_tensor(u[:, :kw2], ta[:, :kw2], pb[:, :kw2],
                                        op=mybir.AluOpType.max)
                for q in range(k):
                    s = gbase + q
                    o = 2 * w * q
                    keyt = spool.tile([128, 256], _F32, tag="key", name="keyt")
                    if PROFILE.get("no_custom"):
                        nc.vector.tensor_tensor(keyt[:, :w], u[:, o : o + w],
                                                u[:, o + w : o + 2 * w],
                                                op=mybir.AluOpType.max)
                        nc.vector.tensor_reduce(zmin[:, s : s + 1], keyt[:, :w],
                                                axis=mybir.AxisListType.X,
                                                op=mybir.AluOpType.min)
                    else:
                        nc.vector._custom_dve(
                            op,
                            out=keyt[:, :w],
                            in0=u[:, o : o + w],
                            in1=u[:, o + w : o + 2 * w],
                            s0=_CLAMP,
                            accum_out=zmin[:, s : s + 1],
                        )
                gbase += k

            nc.sync.dma_start(out_d.ap(), zmin[:])

    nc.compile()
    return nc


def _get_nc(caps, groups):
    key = (caps, groups)
    if key not in _NC_CACHE:
        _NC_CACHE[key] = _build_nc(caps, groups)
    return _NC_CACHE[key]


def _planes64(vertices, faces):
    """Full-precision planes on basis (j, i, 1): [B, 4, 3, F] f64 + aux."""
    v64 = vertices.astype(np.float64)
    fidx = np.asarray(faces).astype(np.int64).reshape(-1)
    fv = v64[:, fidx, :].reshape(_B, _F, 3, 3)
    x0, y0, z0 = fv[:, :, 0, 0], fv[:, :, 0, 1], fv[:, :, 0, 2]
    x1, y1, z1 = fv[:, :, 1, 0], fv[:, :, 1, 1], fv[:, :, 1, 2]
    x2, y2, z2 = fv[:, :, 2, 0], fv[:, :, 2, 1], fv[:, :, 2, 2]

    # area exactly as the reference computes it (float32 ops)
    v32 = vertices.astype(np.float32)
    fv32 = v32[:, fidx, :].reshape(_B, _F, 3, 3)
    xa, ya = fv32[:, :, 0, 0], fv32[:, :, 0, 1]
    xb, yb = fv32[:, :, 1, 0], fv32[:, :, 1, 1]
    xc, yc = fv32[:, :, 2, 0], fv32[:, :, 2, 1]
    area32 = (xb - xa) * (yc - ya) - (yb - ya) * (xc - xa)
    s = np.sign(area32).astype(np.float64)
    valid = np.abs(area32) > 1e-12

    A0 = -(y2 - y1); B0 = x2 - x1; C0 = (y2 - y1) * x1 - (x2 - x1) * y1
    A1 = -(y0 - y2); B1 = x0 - x2; C1 = (y0 - y2) * x2 - (x0 - x2) * y2
    A2 = -(y1 - y0); B2 = x1 - x0; C2 = (y1 - y0) * x0 - (x1 - x0) * y0

    area64 = np.where(valid, area32.astype(np.float64), 1.0)
    Aw = (z0 * A0 + z1 * A1 + z2 * A2) / area64
    Bw = (z0 * B0 + z1 * B1 + z2 * B2) / area64
    Cw = (z0 * C0 + z1 * C1 + z2 * C2) / area64

    planes = np.zeros((_B, 4, 3, _F), np.float64)
    raw = [
        (-_S * s * A0, -_S * s * B0, _OFF - _S * s * C0),
        (-_S * s * A1, -_S * s * B1, _OFF - _S * s * C1),
        (-_S * s * A2, -_S * s * B2, _OFF - _S * s * C2),
        (Aw, Bw, Cw),
    ]
    for k, (a, b, c) in enumerate(raw):
        a = np.where(valid, a, 0.0)
        b = np.where(valid, b, 0.0)
        c = np.where(valid, c, _BIGC)
        # basis change px = 5j + 2.5, py = 5i + 2.5 -> (j, i, 1)
        planes[:, k, 0] = 5.0 * a
        planes[:, k, 1] = 5.0 * b
        planes[:, k, 2] = 2.5 * a + 2.5 * b + c

    xsmin = fv[..., 0].min(2); xsmax = fv[..., 0].max(2)
    ysmin = fv[..., 1].min(2); ysmax = fv[..., 1].max(2)
    zmin_tri = fv[..., 2].min(2)
    return planes, valid, xsmin, xsmax, ysmin, ysmax, zmin_tri


def _split3(c64):
    hi = c64.astype(_BF16_NP).astype(np.float64)
    mid = (c64 - hi).astype(_BF16_NP).astype(np.float64)
    lo = (c64 - hi - mid).astype(_BF16_NP)
    return hi.astype(_BF16_NP), mid.astype(_BF16_NP), lo


def _prepare(vertices, faces):
    planes, valid, xsmin, xsmax, ysmin, ysmax, zmin_tri = _planes64(vertices, faces)
    ntj = _W // _TJ

    # prune per tile, chunk to <=_WMAX, rank-parity balance across all 8
    # cores (a core may hold tiles of any batch - the coef stream is data)
    core_items = [[] for _ in range(8)]  # items: (batch, tile_t, cand_idx_array)
    all_items = []
    for b in range(_B):
        P = planes[b]
        items = all_items
        for t in range(_NTILE):
            tj, ti = t % ntj, t // ntj
            j0, i0 = tj * _TJ, ti * _TI
            xlo, xhi = 5 * j0 + 2.5, 5 * (j0 + _TJ - 1) + 2.5
            ylo, yhi = 5 * i0 + 2.5, 5 * (i0 + _TI - 1) + 2.5
            cand = np.where(valid[b] & (xsmax[b] >= xlo) & (xsmin[b] <= xhi)
                            & (ysmax[b] >= ylo) & (ysmin[b] <= yhi))[0]
            if len(cand):
                corners = np.array(
                    [[j0, i0, 1], [j0 + _TJ - 1, i0, 1],
                     [j0, i0 + _TI - 1, 1], [j0 + _TJ - 1, i0 + _TI - 1, 1]],
                    np.float64)
                Wc = corners @ P[3][:, cand]
                zlo = np.maximum(Wc.min(0), zmin_tri[b][cand])
                covers = np.ones(len(cand), bool)
                for k in range(3):
                    Pc = corners @ P[k][:, cand]
                    covers &= (Pc <= _OFF - _S * _COVER_MARGIN).all(axis=0)
                bound = (Wc.max(0)[covers].min() + _BOUND_MARGIN) if covers.any() else np.inf
                keep = zlo <= bound
                order = cand[keep][np.argsort(zlo[keep])]
            else:
                order = cand
            if len(order) == 0:
                items.append((b, t, order))
            else:
                for c0 in range(0, len(order), _WMAX):
                    items.append((b, t, order[c0 : c0 + _WMAX]))
    all_items.sort(key=lambda it: -len(it[2]))
    for r, it in enumerate(all_items):
        core_items[r % 8].append(it)

    nslot = max(len(ci) for ci in core_items)
    rawcaps = []
    for s in range(nslot):
        m = max((len(ci[s][2]) if s < len(ci) else 0) for ci in core_items)
        rawcaps.append(max(16, ((m + 15) // 16) * 16))

    # groups of consecutive slots padded to the group's (max) width, with
    # pair-merged group width 2*k*w <= 512 (one PSUM bank)
    groups = []
    s = 0
    while s < nslot:
        w = rawcaps[s]
        k = 1
        while s + k < nslot and 2 * (k + 1) * w <= 512:
            k += 1
        groups.append((w, k))
        s += k
    groups = tuple(groups)
    caps = []
    for w, k in groups:
        caps.extend([w] * k)
    caps = tuple(caps)
    total2 = 2 * sum(caps)

    in_maps = []
    for c in range(8):
        items = core_items[c]
        compA = np.zeros((3, total2), np.float64)
        compB = np.zeros((3, total2), np.float64)
        compA[2, :] = _BIGC
        compB[2, :] = _BIGC
        pix_g = np.zeros((3, nslot * 128), np.float32)
        off = 0
        for s in range(nslot):
            w = caps[s]
            jj = ii = np.zeros(128, np.float32)
            if s < len(items):
                b, t, idx = items[s]
                n = len(idx)
                compA[:, off : off + n] = planes[b, 0][:, idx]          # P0
                compA[:, off + w : off + w + n] = planes[b, 3][:, idx]  # W
                compB[:, off : off + n] = planes[b, 1][:, idx]          # P1
                compB[:, off + w : off + w + n] = planes[b, 2][:, idx]  # P2
                tj, ti = t % ntj, t // ntj
                j0, i0 = tj * _TJ, ti * _TI
                jj = j0 + np.tile(np.arange(_TJ, dtype=np.float32), _TI)
                ii = i0 + np.repeat(np.arange(_TI, dtype=np.float32), _TJ)
            off += 2 * w
            pix_g[0, s * 128 : (s + 1) * 128] = jj
            pix_g[1, s * 128 : (s + 1) * 128] = ii
            pix_g[2, s * 128 : (s + 1) * 128] = 1.0
        data = np.zeros((128, total2 + nslot * 128), _BF16_NP)
        for comp, bases in ((compA, (0, 64)), (compB, (32, 96))):
            hi, mid, lo = _split3(comp)
            for base in bases:
                data[base + 0 : base + 3, :total2] = hi
                data[base + 3 : base + 6, :total2] = mid
                data[base + 6 : base + 9, :total2] = lo
        pix16 = np.vstack([pix_g, pix_g, pix_g]).astype(_BF16_NP)
        for base in (0, 32, 64, 96):
            data[base : base + 9, total2:] = pix16
        in_maps.append({"data": data})
    return caps, groups, in_maps, core_items


def kernel(vertices, faces):
    vertices = np.asarray(vertices)
    faces = np.asarray(faces)
    caps, groups, in_maps, core_items = _prepare(vertices, faces)

    nc = _get_nc(caps, groups)
    kw = dict(PROFILE.get("run_kwargs", {}))
    res = run_bass_kernel_spmd(nc, in_maps, list(range(8)), **kw)
    PROFILE["last_result"] = res

    ntj = _W // _TJ
    out = np.full((_B, _H, _W), _CLAMP, np.float32)
    for c in range(8):
        z = res.results[c]["out"]  # [128, nslot]
        for s, (b, t, idx) in enumerate(core_items[c]):
            tj, ti = t % ntj, t // ntj
            j0, i0 = tj * _TJ, ti * _TI
            blk = z[:, s].reshape(_TI, _TJ)
            out[b, i0 : i0 + _TI, j0 : j0 + _TJ] = np.minimum(
                out[b, i0 : i0 + _TI, j0 : j0 + _TJ], blk)
    return out
